# revision 1
# baseline (speedup 1.0000x reference)
"""Trainium2 Bass kernel for nn_CriticOld (twin-Q GNN critic: per-sample kNN +
EdgeConv + MLP head), data-parallel over batch across 8 NeuronCores.

Layout (per core): 128 "problems" on partitions/columns = 2 Q-networks x 64
samples. col = prob*30 + node. EdgeConv is factorized as
    h[p,i,s] = relu(U[p,i] + V[p, idx[p,i,s]]),   U = x(Wa1-Wa2)^T + ba,
    V = x Wa2^T,  out = max_s (h Wb^T + bb)
V rows are written to an HBM table as [bf16_hi | bf16_lo] planes (512B rows)
and gathered transposed (feature-on-partition) via GPSIMD dma_gather; the
U-broadcast add and hi+lo recombine ride the PE as identity-matmul PSUM
accumulations; ScalarE applies ReLU; VectorE does kNN/top-15 (max8 rounds)
and the slot-max reduction from PSUM.
"""
import sys

sys.path.insert(0, "/opt/trn_rl_repo")

import numpy as np
import ml_dtypes

import concourse.bass as bass
from concourse import bacc
import concourse.mybir as mybir
import concourse.tile as tile
from concourse import library_config
from concourse.bass_utils import run_bass_kernel_spmd
from concourse.vector_clock import ScopedClock

f32 = mybir.dt.float32
f32r = mybir.dt.float32r
bf16 = mybir.dt.bfloat16
u16 = mybir.dt.uint16
i16 = mybir.dt.int16
AF = mybir.ActivationFunctionType
OP = mybir.AluOpType
AX = mybir.AxisListType

BS, N, K, HID, EMB = 512, 30, 15, 128, 64
CORES = 8
BSC = BS // CORES          # samples per core
P = 2 * BSC                # 128 problems per core (2 Q-nets)
COLS = P * N               # 3840
# phase-D chunking: per b-block (16 probs), 7 chunks of 4 node-groups (960
# real pair-cols, num_idxs padded to 1024) + 1 chunk of 2 node-groups (480
# real, padded 512). dma_gather fails above ~1008 valid idxs per call.
CHUNKS = []                # (b, i_base, n_i)
for _b in range(8):
    for _j in range(7):
        CHUNKS.append((_b, 4 * _j, 4))
    CHUNKS.append((_b, 28, 2))
MB = 480                   # idx m-cols per b-block in idx_sb

import os
ABL = os.environ.get("KABL", "")  # ablation switches: noreduce,noid,nogather,noknn


def _patched_drain_and_barrier(self, tick_clock, wait_clock):
    # this walrus build caps sync-waits at 1/instruction; spread the
    # end-of-kernel waits over SP NOPs instead of one multi-wait Drain.
    nc = self.nc
    probe = nc.sync.nop()
    wait_clock.add_sem_waits(probe.ins, ScopedClock({None: tick_clock.global_clock}))
    si = probe.ins.sync_info
    waits = list(si.on_wait) if si is not None else []
    if len(waits) > 1:
        si.on_wait = [waits[0]]
        for w in waits[1:]:
            extra = nc.sync.nop()
            esi = extra.ins.sync_info
            if esi is None:
                extra.ins.sync_info = mybir.SyncInfo(on_wait=[w], on_update=[])
            else:
                esi.on_wait = [w]
    nc.sync.drain()
    nc.all_engine_barrier()
    assert self.sems is not None
    popped = nc._tile_sem_poison_stack.pop()
    assert popped is self._sem_poison
    nc.clear_and_free_semaphores(list(self.sems.allocated().values()))
    nc.all_engine_barrier()


tile.TileContext._drain_and_barrier = _patched_drain_and_barrier

_ws_cnt = [0]


def split_excess_waits(nc):
    """This walrus build supports at most 1 sync-wait per instruction (2 for
    EventSemaphore). Tile attaches several; move the extras onto same-engine
    NOPs inserted right before the instruction."""
    for fn in nc.m.functions:
        for bb in fn.blocks:
            new_list = []
            for inst in bb.instructions:
                si = inst.sync_info
                cap = 2 if isinstance(inst, mybir.InstEventSemaphore) else 1
                if si is not None and si.on_wait is not None and len(si.on_wait) > cap:
                    waits = list(si.on_wait)
                    for w in waits[:-cap]:
                        n = mybir.InstNoOp(name=f"I-wsplit-{_ws_cnt[0]}", ins=[], outs=[])
                        _ws_cnt[0] += 1
                        n.engine = inst.engine
                        n.sync_info = mybir.SyncInfo(on_wait=[w], on_update=[])
                        nc.register_instruction(n, overwrite=True)
                        new_list.append(n)
                    si.on_wait = waits[-cap:]
                new_list.append(inst)
            bb.instructions[:] = new_list


def ap_of(t, offset, dims):
    """Build a raw AP on tile/dram tensor t: dims = [[step, count], ...] (elements)."""
    base = t.ap() if hasattr(t, "ap") and not isinstance(t, bass.AP) else t
    return bass.AP(tensor=base.tensor, offset=base.offset + offset, ap=dims)


def build_program():
    nc = bacc.Bacc("TRN2", debug=False)

    din = {}
    def inp(name, shape, dtype=f32):
        din[name] = nc.dram_tensor(name, shape, dtype, kind="ExternalInput")
        return din[name]

    inp("x_T", [4, COLS])
    inp("x_pp", [P, N * 4])
    inp("wm1aT", [2 * (HID + EMB + 2), HID])   # (388, 128)
    inp("wm1bT", [HID, HID])
    inp("wi1T", [4, HID]); inp("wi2T", [4, HID])
    inp("emb1T", [EMB, 3]); inp("emb2T", [EMB, 3])
    inp("wca1T", [HID, HID]); inp("wca2T", [HID, HID])
    inp("wcb1T", [HID, 1]); inp("wcb2T", [HID, 1])
    inp("bi1", [HID, 1]); inp("bi2", [HID, 1])
    inp("bm1a", [HID, 1]); inp("bm1b", [HID, 1])
    inp("bca1", [HID, 1]); inp("bca2", [HID, 1])
    inp("bcb1", [1, 1]); inp("bcb2", [1, 1])
    inp("iden", [HID, HID])
    inp("idenb", [HID, HID], bf16)
    inp("p30", [P, 1])

    vtab = nc.dram_tensor("vtab", [COLS, 256], u16)        # internal
    idx_dram = nc.dram_tensor("idx_dram", [P, N * K], i16)  # internal
    qout = nc.dram_tensor("qout", [1, COLS], f32, kind="ExternalOutput")

    from contextlib import ExitStack
    ctx = ExitStack()
    with tile.TileContext(nc) as tc, ctx:
        consts = ctx.enter_context(tc.tile_pool(name="consts", bufs=1))
        big = ctx.enter_context(tc.tile_pool(name="big", bufs=1))
        knn = ctx.enter_context(tc.tile_pool(name="knn", bufs=1))
        t8p = ctx.enter_context(tc.tile_pool(name="t8p", bufs=8))
        vrow = ctx.enter_context(tc.tile_pool(name="vrow", bufs=6))
        gpool = ctx.enter_context(tc.tile_pool(name="gpool", bufs=6))
        hpool = ctx.enter_context(tc.tile_pool(name="hpool", bufs=6))
        ps_m = ctx.enter_context(tc.tile_pool(name="ps_m", bufs=1, space="PSUM"))
        ps_pre = ctx.enter_context(tc.tile_pool(name="ps_pre", bufs=4, space="PSUM"))
        ps_2 = ctx.enter_context(tc.tile_pool(name="ps_2", bufs=3, space="PSUM"))

        nc.gpsimd.load_library(library_config.mlp)

        # ---- load inputs to SBUF ----
        sb = {}
        for name, parts, width, dt in (
            ("x_T", 4, COLS, f32), ("x_pp", P, N * 4, f32),
            ("wm1bT", HID, HID, f32r),
            ("wi1T", 4, HID, f32), ("wi2T", 4, HID, f32),
            ("emb1T", EMB, 3, f32), ("emb2T", EMB, 3, f32),
            ("wca1T", HID, HID, f32), ("wca2T", HID, HID, f32),
            ("wcb1T", HID, 1, f32), ("wcb2T", HID, 1, f32),
            ("bi1", HID, 1, f32), ("bi2", HID, 1, f32),
            ("bm1a", HID, 1, f32), ("bm1b", HID, 1, f32),
            ("bca1", HID, 1, f32), ("bca2", HID, 1, f32),
            ("bcb1", 1, 1, f32), ("bcb2", 1, 1, f32),
            ("iden", HID, HID, f32r), ("idenb", HID, HID, bf16),
            ("p30", P, 1, f32),
        ):
            t = consts.tile([parts, width], dt, tag=name)
            src = din[name].ap()
            if dt == f32r:
                src = src.bitcast(f32r)
            nc.sync.dma_start(out=t[:], in_=src)
            sb[name] = t
        # wm1aT slices
        wa_sl = {}
        for nm, lo, hi in (("a1a", 0, 128), ("a1b", 128, 194),
                           ("a2a", 194, 322), ("a2b", 322, 388)):
            t = consts.tile([hi - lo, HID], f32, tag="wa_" + nm)
            nc.sync.dma_start(out=t[:], in_=din["wm1aT"].ap()[lo:hi, :])
            wa_sl[nm] = t
        wau1 = consts.tile([128, HID], f32)
        wau2 = consts.tile([66, HID], f32)
        nc.vector.tensor_sub(wau1[:], wa_sl["a1a"][:], wa_sl["a2a"][:])
        nc.vector.tensor_sub(wau2[:], wa_sl["a1b"][:], wa_sl["a2b"][:])

        # ---- phase A: init features (feature-on-partition) ----
        initT1 = big.tile([HID, COLS], f32, tag="bigA")
        initT2 = big.tile([66, COLS], f32)
        nc.vector.memset(initT2[64:66, :], 0.0)
        clsr = {}
        for q in (0, 1):
            t = consts.tile([EMB, 3], f32, tag=f"clsr{q}")
            nc.scalar.activation(t[:], sb["emb1T" if q == 0 else "emb2T"][:], AF.Relu)
            clsr[q] = t
        for q in (0, 1):
            # initT2 rows 0..63 <- cls columns by category (i//10), bcast over prob
            dst = bass.AP(tensor=initT2.tensor, offset=initT2.offset + q * (COLS // 2),
                          ap=[[initT2.ap[0][0], 64], [N, BSC], [10, 3], [1, 10]])
            src = bass.AP(tensor=clsr[q].tensor, offset=clsr[q].offset,
                          ap=[[clsr[q].ap[0][0], 64], [0, BSC], [1, 3], [0, 10]])
            nc.vector.tensor_copy(dst, src)
        for c in range(8):
            q = 0 if c < 4 else 1
            ps = ps_m.tile([HID, 480], f32)
            nc.tensor.matmul(ps[:], lhsT=sb["wi1T" if q == 0 else "wi2T"][:],
                             rhs=sb["x_T"][:, c * 480:(c + 1) * 480],
                             start=True, stop=True)
            nc.scalar.activation(initT1[:, c * 480:(c + 1) * 480], ps[:], AF.Relu,
                                 bias=sb["bi1" if q == 0 else "bi2"][:])

        # ---- phase B: U^T (+ba) ----
        UT = big.tile([HID, COLS], f32r)
        for c in range(8):
            ps = ps_m.tile([HID, 480], f32)
            sl = slice(c * 480, (c + 1) * 480)
            nc.tensor.matmul(ps[:], lhsT=wau1[:],
                             rhs=initT1[:, sl], start=True, stop=False)
            nc.tensor.matmul(ps[:], lhsT=wau2[:],
                             rhs=initT2[:, sl], start=False, stop=True)
            nc.scalar.activation(UT[:, sl], ps[:], AF.Copy)

        # ---- phase B2: V rows -> HBM table [hi|lo] ----
        for blk in range(N):
            ps = ps_m.tile([128, 128], f32)
            sl = slice(blk * 128, (blk + 1) * 128)
            nc.tensor.matmul(ps[:], lhsT=initT1[:, sl], rhs=wa_sl["a2a"][:],
                             start=True, stop=False)
            nc.tensor.matmul(ps[:], lhsT=initT2[:, sl], rhs=wa_sl["a2b"][:],
                             start=False, stop=True)
            vhi = vrow.tile([128, 128], bf16)
            nc.scalar.activation(vhi[:], ps[:], AF.Copy)
            vlo = vrow.tile([128, 128], bf16)
            nc.vector.tensor_sub(vlo[:], ps[:], vhi[:])
            nc.sync.dma_start(out=vtab.ap()[sl, 0:128], in_=vhi[:].bitcast(u16))
            nc.sync.dma_start(out=vtab.ap()[sl, 128:256], in_=vlo[:].bitcast(u16))

        # ---- phase C: kNN + top-15 ----
        diff = knn.tile([P, 3600], f32)
        xpp = sb["x_pp"]
        nc.vector.tensor_sub(
            diff[:],
            ap_of(xpp, 0, [list(xpp.ap[0]), [4, N], [0, N], [1, 4]]),
            ap_of(xpp, 0, [list(xpp.ap[0]), [0, N], [4, N], [1, 4]]))
        negsq = knn.tile([P, 3600], f32)
        nc.vector.scalar_tensor_tensor(out=negsq[:], in0=diff[:], scalar=-1.0,
                                       in1=diff[:], op0=OP.mult, op1=OP.mult)
        negd = knn.tile([P, 900], f32)
        nc.vector.tensor_reduce(
            out=negd[:], in_=ap_of(negsq, 0, [list(negsq.ap[0]), [4, 900], [1, 4]]),
            axis=AX.X, op=OP.add)
        nc.vector.memset(ap_of(negd, 0, [list(negd.ap[0]), [31, N]]), -1e30)

        idxall = knn.tile([P, 480], u16)
        if "noknn" in ABL:
            nc.vector.memset(idxall[:], 0)
        for i in ([] if "noknn" in ABL else range(N)):
            nd = negd[:, i * 30:(i + 1) * 30]
            m8 = t8p.tile([P, 8], f32, tag="m8")
            scr = t8p.tile([P, 30], f32, tag="scr")
            m8b = t8p.tile([P, 8], f32, tag="m8b")
            nc.vector.max(m8[:], nd)
            nc.vector.max_index(idxall[:, i * 16:i * 16 + 8], m8[:], nd)
            nc.vector.match_replace(scr[:], in_to_replace=m8[:], in_values=nd,
                                    imm_value=-1e30)
            nc.vector.max(m8b[:], scr[:])
            nc.vector.max_index(idxall[:, i * 16 + 8:i * 16 + 16], m8b[:], scr[:])

        gfp = knn.tile([P, N * K], f32)
        nc.vector.tensor_scalar(
            out=ap_of(gfp, 0, [list(gfp.ap[0]), [15, N], [1, 8]]),
            in0=ap_of(idxall, 0, [list(idxall.ap[0]), [16, N], [1, 8]]),
            scalar1=sb["p30"][:], scalar2=None, op0=OP.add)
        nc.vector.tensor_scalar(
            out=ap_of(gfp, 8, [list(gfp.ap[0]), [15, N], [1, 7]]),
            in0=ap_of(idxall, 8, [list(idxall.ap[0]), [16, N], [1, 7]]),
            scalar1=sb["p30"][:], scalar2=None, op0=OP.add)
        gi16 = knn.tile([P, N * K], i16)
        nc.vector.tensor_copy(gi16[:], gfp[:])
        nc.sync.dma_start(out=idx_dram.ap(), in_=gi16[:])

        idx_sb = big.tile([P, 8 * MB], i16)
        nc.vector.memset(idx_sb[:], -1)
        for ci, (b, i_base, n_i) in enumerate(CHUNKS):
            j = i_base // 4 if n_i == 4 else 7
            mo = b * MB + (j * 64 if n_i == 4 else 448)
            nreal_m = n_i * 60 // 4  # 60 for n_i=4, 30 for n_i=2
            src = bass.AP(tensor=idx_dram, offset=(b * 16) * 450 + i_base * 15,
                          ap=[[0, 8], [450, 16], [1, n_i * 15]])
            nc.sync.dma_start(out=idx_sb[:, mo: mo + nreal_m], in_=src)

        # ---- phase D: gather + pair build + mm2 + slot-max ----
        Hfin = big.tile([HID, COLS], f32)
        nreal_big = nc.gpsimd.snap(960)
        nreal_small = nc.gpsimd.snap(480)
        for ci, (b, i_base, n_i) in enumerate(CHUNKS):
            j = i_base // 4 if n_i == 4 else 7
            mo = b * MB + (j * 64 if n_i == 4 else 448)
            nidx_c = 1024 if n_i == 4 else 512
            gout = gpool.tile([P, 2 * 1024], u16, tag="gout")
            if "nogather" in ABL:
                nc.vector.memset(gout[:, 0:1], 0)
            else:
                nc.gpsimd.dma_gather(
                    out_ap=gout[:, 0:2 * nidx_c].rearrange("p (c n) -> p c n", c=2),
                    in_ap=vtab.ap()[b * 480:(b + 1) * 480, :],
                    idxs_ap=idx_sb[:, mo: mo + nidx_c // 16],
                    num_idxs=nidx_c,
                    num_idxs_reg=nreal_big if n_i == 4 else nreal_small,
                    elem_size=256, transpose=True)
            # group same-weight matmuls so the PE reloads its stationary
            # operand 3x per chunk instead of ~6x (hi0,hi1 | lo0,lo1 | U0,U1
            # | mm2_0,mm2_1). Accumulation groups interleave across the two
            # PSUM banks, which is legal; order within a group is preserved.
            nk = n_i // 2
            psps = [ps_pre.tile([HID, 480], f32, tag="psp", name=f"psp_{ci}_{_k}") for _k in range(nk)]
            for k in range(nk):
                nc.tensor.matmul(psps[k][:], lhsT=sb["idenb"][:],
                                 rhs=gout[:, k * 480:(k + 1) * 480].bitcast(bf16),
                                 start=True, stop=False)
            for k in range(nk):
                nc.tensor.matmul(psps[k][:], lhsT=sb["idenb"][:],
                                 rhs=gout[:, nidx_c + k * 480:nidx_c + (k + 1) * 480].bitcast(bf16),
                                 start=False, stop=False)
            for k in range(nk):
                # U cols (i2, s, q): col = (b*16+q)*30 + i_base + 2k + i2
                uap = ap_of(UT, b * 480 + i_base + 2 * k,
                            [list(UT.ap[0]), [1, 2], [0, K], [30, 16]]).bitcast(f32r)
                nc.tensor.matmul(psps[k][:], lhsT=sb["iden"][:], rhs=uap,
                                 start=False, stop=True)
            hs = []
            for k in range(nk):
                h = hpool.tile([HID, 480], f32r, tag="h")
                nc.scalar.activation(h[:], psps[k][:], AF.Relu, bias=sb["bm1a"][:])
                hs.append(h)
            ps2s = [ps_2.tile([HID, 480], f32, tag="ps2", name=f"ps2_{ci}_{_k}") for _k in range(nk)]
            for k in range(nk):
                nc.tensor.matmul(ps2s[k][:], lhsT=sb["wm1bT"][:],
                                 rhs=hs[k][:], start=True, stop=True)
            for k in range(nk):
                nc.vector.tensor_reduce(
                    out=Hfin[:, b * 480 + (i_base + 2 * k) * 16:
                             b * 480 + (i_base + 2 * k) * 16 + 32],
                    in_=ap_of(ps2s[k], 0, [list(ps2s[k].ap[0]), [240, 2], [1, 16], [16, 15]]),
                    axis=AX.X, op=OP.max)

        # ---- head ----
        Hb = big.tile([HID, COLS], f32, tag="bigA")  # reuse initT1 slot
        nc.scalar.activation(Hb[:], Hfin[:], AF.Relu, bias=sb["bm1b"][:])
        HC = big.tile([HID, COLS], f32)
        for c in range(8):
            q = 0 if c < 4 else 1
            sl = slice(c * 480, (c + 1) * 480)
            ps = ps_m.tile([HID, 480], f32)
            nc.tensor.matmul(ps[:], lhsT=sb["wca1T" if q == 0 else "wca2T"][:],
                             rhs=Hb[:, sl], start=True, stop=True)
            nc.scalar.activation(HC[:, sl], ps[:], AF.Relu,
                                 bias=sb["bca1" if q == 0 else "bca2"][:])
        qrow = big.tile([1, COLS], f32)
        for c in range(8):
            q = 0 if c < 4 else 1
            sl = slice(c * 480, (c + 1) * 480)
            ps = ps_m.tile([1, 480], f32, tag="ps")
            nc.tensor.matmul(ps[:], lhsT=sb["wcb1T" if q == 0 else "wcb2T"][:],
                             rhs=HC[:, sl], start=True, stop=True)
            nc.vector.tensor_scalar(out=qrow[:, sl], in0=ps[:],
                                    scalar1=sb["bcb1" if q == 0 else "bcb2"][:],
                                    scalar2=None, op0=OP.add)
        nc.sync.dma_start(out=qout.ap(), in_=qrow[:])

    nc.compile()
    split_excess_waits(nc)
    return nc


_CACHED = {}


def _get_program():
    if "nc" not in _CACHED:
        _CACHED["nc"] = build_program()
    return _CACHED["nc"]


def _host_inputs(state, action, weights):
    nodes1 = np.concatenate(
        [state.reshape(BS, N, 2), action.reshape(BS, N, 2)], axis=-1)
    nodes2 = np.concatenate([state, action], axis=1).reshape(BS, N, 4)
    iden = np.eye(HID, dtype=np.float32)
    shared = {
        "wm1aT": np.ascontiguousarray(weights["W_m1a"].T),
        "wm1bT": np.ascontiguousarray(weights["W_m1b"].T),
        "wi1T": np.ascontiguousarray(weights["W_init1"].T),
        "wi2T": np.ascontiguousarray(weights["W_init2"].T),
        "emb1T": np.ascontiguousarray(weights["emb1"].T),
        "emb2T": np.ascontiguousarray(weights["emb2"].T),
        "wca1T": np.ascontiguousarray(weights["W_c1a"].T),
        "wca2T": np.ascontiguousarray(weights["W_c2a"].T),
        "wcb1T": np.ascontiguousarray(weights["W_c1b"].T),
        "wcb2T": np.ascontiguousarray(weights["W_c2b"].T),
        "bi1": weights["b_init1"].reshape(HID, 1),
        "bi2": weights["b_init2"].reshape(HID, 1),
        "bm1a": weights["b_m1a"].reshape(HID, 1),
        "bm1b": weights["b_m1b"].reshape(HID, 1),
        "bca1": weights["b_c1a"].reshape(HID, 1),
        "bca2": weights["b_c2a"].reshape(HID, 1),
        "bcb1": weights["b_c1b"].reshape(1, 1),
        "bcb2": weights["b_c2b"].reshape(1, 1),
        "iden": iden,
        "idenb": iden.astype(ml_dtypes.bfloat16),
        "p30": ((np.arange(P, dtype=np.float32) % 16) * N).reshape(P, 1),
    }
    shared = {k: np.ascontiguousarray(v, dtype=v.dtype) for k, v in shared.items()}
    in_maps = []
    for c in range(CORES):
        x_pp = np.concatenate(
            [nodes1[c * BSC:(c + 1) * BSC], nodes2[c * BSC:(c + 1) * BSC]], axis=0)
        x_T = np.ascontiguousarray(x_pp.transpose(2, 0, 1).reshape(4, COLS))
        m = dict(shared)
        m["x_pp"] = np.ascontiguousarray(x_pp.reshape(P, N * 4))
        m["x_T"] = x_T
        in_maps.append(m)
    return in_maps


def kernel(**inputs):
    state = np.asarray(inputs["state"], np.float32)
    action = np.asarray(inputs["action"], np.float32)
    weights = {k: np.asarray(v, np.float32) for k, v in inputs.items()
               if k not in ("state", "action")}
    nc = _get_program()
    in_maps = _host_inputs(state, action, weights)
    res = run_bass_kernel_spmd(nc, in_maps, core_ids=list(range(CORES)))
    q1 = np.zeros((BS, N), np.float32)
    q2 = np.zeros((BS, N), np.float32)
    for c in range(CORES):
        arr = res.results[c]["qout"].reshape(8, N, 16)  # (b, i, q)
        probs = arr.transpose(0, 2, 1).reshape(P, N)    # prob = b*16+q
        q1[c * BSC:(c + 1) * BSC] = probs[:BSC]
        q2[c * BSC:(c + 1) * BSC] = probs[BSC:]
    return (q1, q2)


if __name__ == "__main__":
    rng = np.random.default_rng(0)
    ins = {"state": rng.standard_normal((BS, 120)).astype(np.float32)[:, :60],
           "action": rng.standard_normal((BS, 60)).astype(np.float32)}
    print("smoke build only")
    build_program()
    print("built ok")

